# revision 1
# baseline (speedup 1.0000x reference)
"""Trainium2 Bass kernel for nn_MultiHeadAttention_32031866093611.

Sharding: pure data parallel — batch b -> NeuronCore b (B == n_cores == 8).
Weights replicated. No collectives.

Per-core program (batch b, S=1024, D=1024, H=16, DK=64), all matmuls fp32r:

  inputs (per core): xT = x[b].T [D, S], Wq/Wk/Wv/Wo [D, D] (as stored),
                     bq/bk/bv/bo [1, D], masks (host-built from prefix[b]).

  qT[c]   = (Wq[:, c*128:+128]).T @ xT + bq       -> [128 d', 1024 s]   (8 chunks)
  kT[c]   = same with Wk                          -> [128 d', 1024 s]
  v[sc]   = (xT[:, sc*128:+128]).T @ Wv + bv      -> [128 s, 16, 64+1]  (ones col)
  per head h (c=h//2, r=h%2*64):
    for kc in 0..7:
      sT[kc] = kT[c][r:r+64, kc*128:+128].T @ qT[c][r:r+64, :]   # [128 k, 1024 q]
      sT[kc] += diag/column additive masks (DVE, on cols >= kc*128)
      eT[kc] = exp(sT[kc])                                        # ACT, psum->sbuf
      outT  += v[kc][:, h, :].T @ eT[kc]       # [65, 1024]: row 64 = softmax denom
    attnT[c][r:r+64, :] = outT[0:64, :] * bcast(1/outT[64, :])
  out[sc] = (attnT[.][:, sc*128:+128]).T @ Wo + bo  -> [128 s, 1024 d] -> DRAM

The mask allowed(q,k) = (q < prefix) | (k >= q) decomposes in the transposed
[k, q] tile grid as: blocks kc > qc fully allowed (untouched); everything at or
below the diagonal (cols >= kc*128) gets one multiplicative 0/1 u8 mask applied
to the exp output on DVE (exp(s)*m == exp(s + additive mask)).

Schedule: flat (h, kc) stream with PV matmuls lagging scores/exp by 3 tiles
(in-order PE never waits on a just-issued exp); o_proj chunk k (which only
needs heads 2k, 2k+1 after the interleave) is emitted two heads after head
2k+1 retires, inside the ACT-bound attention phase; ~20 warm-up matmuls keep
the PE HAM clock-gate hot while the first x/Wq DMAs land.
"""

import numpy as np

import concourse.bass as bass
import concourse.mybir as mybir
import concourse.tile as tile
from concourse import bacc
from concourse.bass_utils import run_bass_kernel_spmd

B, S, D, H = 8, 1024, 1024, 16
DK = D // H  # 64
P = 128
NCHUNK = S // P  # 8
NCORES = 8
F32R = mybir.dt.float32r
F32 = mybir.dt.float32
EXP = mybir.ActivationFunctionType.Exp
NEG = -1.0e30
HALF = 512  # fp32 moving-operand max
MSK_OFF = [0]
for _kc in range(1, 8):
    MSK_OFF.append(MSK_OFF[-1] + S - (_kc - 1) * P)

_CACHED = {}


def build_nc(repeats=1):
    nc = bacc.Bacc("TRN2", target_bir_lowering=False, debug=False, num_devices=NCORES)

    xt_d = nc.dram_tensor("xt", [D, S], F32R, kind="ExternalInput").ap()
    wq_d = nc.dram_tensor("wq", [D, D], F32R, kind="ExternalInput").ap()
    wk_d = nc.dram_tensor("wk", [D, D], F32R, kind="ExternalInput").ap()
    wv_d = nc.dram_tensor("wv", [D, D], F32R, kind="ExternalInput").ap()
    wo_d = nc.dram_tensor("wo", [D, D], F32R, kind="ExternalInput").ap()
    bqk_d = nc.dram_tensor("bqk", [P, 2 * NCHUNK], F32, kind="ExternalInput").ap()
    ones_d = nc.dram_tensor("ones2d", [P, P], F32R, kind="ExternalInput").ap()
    bv_d = nc.dram_tensor("bv", [P, D], F32, kind="ExternalInput").ap()
    bo_d = nc.dram_tensor("bo", [P, D], F32, kind="ExternalInput").ap()
    msk_d = nc.dram_tensor("mask8", [P, 4608], mybir.dt.uint8, kind="ExternalInput").ap()
    out_d = nc.dram_tensor("out", [S, D], F32, kind="ExternalOutput").ap()

    with tile.TileContext(nc) as tc:
        with (
            tc.tile_pool(name="w", bufs=18) as wpool,
            tc.tile_pool(name="big", bufs=2) as bigpool,
            tc.tile_pool(name="qk", bufs=8) as qkpool,
            tc.tile_pool(name="v", bufs=8) as vpool,
            tc.tile_pool(name="cst", bufs=1) as cstpool,
            tc.tile_pool(name="exp", bufs=5) as exppool,
            tc.tile_pool(name="rcp", bufs=1) as rcppool,
            tc.tile_pool(name="rbc", bufs=1) as rbcpool,
            tc.tile_pool(name="osb", bufs=1) as osbpool,
            tc.tile_pool(name="pp", bufs=2, space="PSUM") as pp,
            tc.tile_pool(name="po", bufs=2, space="PSUM") as po,
        ):
            for _rep in range(repeats):
                # ---- x chunks + Wq strips interleaved (fast PE start), cst after ----
                ones2d = cstpool.tile([P, P], F32R, tag="ones2d")
                nc.sync.dma_start(ones2d[:], ones_d[:])
                ones = ones2d[0:1, :]
                xtq = [
                    bigpool.tile([P, 4, S], F32R, tag="big", name=f"xtq_{g}")
                    for g in range(2)
                ]
                def whalf(nm, w_dram, hf):
                    """8 half-strips [128, 512] of W columns [hf*512, (hf+1)*512)."""
                    ts = [
                        wpool.tile([P, HALF], F32R, tag="w", name=f"{nm}{hf}_{dc}")
                        for dc in range(NCHUNK)
                    ]
                    sl = slice(hf * HALF, (hf + 1) * HALF)
                    for dc in range(NCHUNK):
                        nc.sync.dma_start(ts[dc][:], w_dram[dc * P : (dc + 1) * P, sl])
                    return ts

                for dc in range(NCHUNK):
                    nc.sync.dma_start(
                        xtq[dc // 4][:, dc % 4, 0:HALF],
                        xt_d[dc * P : (dc + 1) * P, 0:HALF],
                    )
                qh0 = whalf("wq", wq_d, 0)
                for dc in range(NCHUNK):
                    nc.sync.dma_start(
                        xtq[dc // 4][:, dc % 4, HALF:S],
                        xt_d[dc * P : (dc + 1) * P, HALF:S],
                    )
                kh0 = whalf("wk", wk_d, 0)
                xt = [xtq[dc // 4][:, dc % 4, :] for dc in range(NCHUNK)]

                # PE warm-up: ~20 throwaway matmuls on the first-arriving tiny
                # tile keep the HAM clock-gate busy while x/Wq stream in.
                wps = pp.tile([P, S], F32, tag="pp", name="warmup_ps")
                for wi in range(18):
                    nc.tensor.matmul(
                        wps[:, 0:P], ones2d[:], ones2d[:], start=True, stop=True
                    )
                bqk = cstpool.tile([P, 2 * NCHUNK], F32, tag="bqk")
                nc.sync.dma_start(bqk[:], bqk_d[:])
                msk = cstpool.tile([P, 4608], mybir.dt.uint8, tag="msk")
                nc.sync.dma_start(msk[:], msk_d[:])
                bias = {}
                # bv (v-proj) and bo (o_proj) lifetimes don't overlap: share slot
                bias["bv"] = cstpool.tile([P, D], F32, tag="bvbo", name="bv_bc")
                nc.sync.dma_start(bias["bv"][:], bv_d[:])

                # ---- helper: dense [d', s] projection (qT / kT) ----
                def proj_half(whalf_tiles, chalf, bcol0, out_tag):
                    """qT/kT chunks chalf*4 .. chalf*4+3 from one W column half."""
                    outs = []
                    for cp in range(2):
                        cs = (chalf * 4 + 2 * cp, chalf * 4 + 2 * cp + 1)
                        pss = {
                            c: pp.tile([P, S], F32, tag="pp", name=f"ps_{out_tag}_{c}")
                            for c in cs
                        }
                        for j in range(2):
                            sl = slice(j * HALF, (j + 1) * HALF)
                            for c in cs:
                                lc = (c % 4) * P
                                for dc in range(NCHUNK):
                                    nc.tensor.matmul(
                                        pss[c][:, sl],
                                        whalf_tiles[dc][:, lc : lc + P],
                                        xt[dc][:, sl],
                                        start=(dc == 0),
                                        stop=(dc == NCHUNK - 1),
                                    )
                        for c in cs:
                            o = qkpool.tile(
                                [P, S], F32R, tag=out_tag, name=f"{out_tag}_{c}"
                            )
                            nc.vector.tensor_add(
                                o[:],
                                pss[c][:],
                                bqk[:, bcol0 + c : bcol0 + c + 1].to_broadcast((P, S)),
                            )
                            outs.append(o)
                    return outs

                with nc.named_scope("qk_proj"):
                    qT = proj_half(qh0, 0, 0, "qT")
                    qh1 = whalf("wq", wq_d, 1)
                    kT = proj_half(kh0, 0, NCHUNK, "kT")
                    kh1 = whalf("wk", wk_d, 1)
                    qT += proj_half(qh1, 1, 0, "qT")
                    kT += proj_half(kh1, 1, NCHUNK, "kT")

                # ---- v projection: [s, 16, 65] with ones column ----
                with nc.named_scope("v_proj"):
                    vh = [whalf("wv", wv_d, 0), whalf("wv", wv_d, 1)]
                    vtiles = []
                    for sc in range(NCHUNK):
                        ps = pp.tile([P, S], F32, tag="pp")
                        for j in range(2):
                            sl = slice(j * HALF, (j + 1) * HALF)
                            for dc in range(NCHUNK):
                                nc.tensor.matmul(
                                    ps[:, sl],
                                    xt[dc][:, sc * P : (sc + 1) * P],
                                    vh[j][dc][:],
                                    start=(dc == 0),
                                    stop=(dc == NCHUNK - 1),
                                )
                        vt = vpool.tile([P, H, DK + 1], F32R, tag="v")
                        nc.vector.tensor_add(
                            vt[:, :, 0:DK],
                            ps[:].rearrange("p (h d) -> p h d", h=H),
                            bias["bv"][:].rearrange("p (h d) -> p h d", h=H),
                        )
                        nc.vector.tensor_copy(
                            vt[:, :, DK : DK + 1], ones2d[:, 0:1].to_broadcast((P, H, 1))
                        )
                        vtiles.append(vt)

                # ---- attention heads ----
                bias["bo"] = cstpool.tile([P, D], F32, tag="bvbo", name="bo_bc")
                nc.sync.dma_start(bias["bo"][:], bo_d[:])
                attn = [None, None]

                # Wo strips prefetched before the head loop (slots free up as
                # Wq/Wk strips retire); o_proj chunk sc only needs heads 2sc,2sc+1.
                oh = [whalf("wo", wo_d, 0), whalf("wo", wo_d, 1)]

                def emit_scores_exp(h, kc):
                    """scores on PE, exp on ACT, multiplicative 0/1 masks on DVE."""
                    c, r = h // 2, (h % 2) * DK
                    pss = pp.tile([P, S], F32, tag="pp", name=f"pss_{h}_{kc}")
                    lhs = kT[c][r : r + DK, kc * P : (kc + 1) * P]
                    for j in range(2):
                        sl = slice(j * HALF, (j + 1) * HALF)
                        nc.tensor.matmul(
                            pss[:, sl],
                            lhs,
                            qT[c][r : r + DK, sl],
                            start=True,
                            stop=True,
                        )
                    et = exppool.tile([P, S], F32R, tag="exp", name=f"et_{h}_{kc}")
                    nc.scalar.activation(et[:], pss[:], EXP)
                    # one 0/1 mask mult over cols [kc*128, 1024): diag pattern on
                    # the diagonal block, column mask below the diagonal
                    w = S - kc * P
                    off = MSK_OFF[kc]
                    nc.vector.tensor_mul(
                        et[:, kc * P : S], et[:, kc * P : S], msk[:, off : off + w]
                    )
                    return et

                def emit_pv(h, kc, pso, et):
                    for j in range(2):
                        sl = slice(j * HALF, (j + 1) * HALF)
                        nc.tensor.matmul(
                            pso[0 : DK + 1, sl],
                            vtiles[kc][:, h, :],
                            et[:, sl],
                            start=(kc == 0),
                            stop=(kc == NCHUNK - 1),
                        )

                def emit_norm(h, pso):
                    rcp = rcppool.tile([1, S], F32, tag="rcp", name=f"rcp_{h}")
                    nc.vector.reciprocal(rcp[:], pso[DK : DK + 1, :])
                    rbc = rbcpool.tile([DK, S], F32, tag="rbc", name=f"rbc_{h}")
                    nc.gpsimd.partition_broadcast(rbc[:], rcp[:])
                    # attn[g][e*64+d, cc, h*64+u] = O_h[u*16 + 2*(4g+cc) + e, d]/denom
                    src = pso[0:DK, :].rearrange("d (u j) -> d j u", j=16)
                    rbs = rbc[:].rearrange("d (u j) -> d j u", j=16)
                    for g in range(2):
                        if attn[g] is None:
                            attn[g] = bigpool.tile(
                                [P, 4, S], F32R, tag="big", name=f"attnq_{g}"
                            )
                        for e in range(2):
                            jsl = slice(8 * g + e, 8 * (g + 1), 2)
                            nc.vector.tensor_mul(
                                attn[g][e * DK : (e + 1) * DK, :, h * DK : (h + 1) * DK],
                                src[:, jsl, :],
                                rbs[:, jsl, :],
                            )

                def emit_oproj(sc):
                    ps = po.tile([P, S], F32, tag="po", name=f"psf_{sc}")
                    for j in range(2):
                        sl = slice(j * HALF, (j + 1) * HALF)
                        for cc in range(NCHUNK):
                            nc.tensor.matmul(
                                ps[:, sl],
                                attn[cc // 4][:, cc % 4, sc * P : (sc + 1) * P],
                                oh[j][cc][:],
                                start=(cc == 0),
                                stop=(cc == NCHUNK - 1),
                            )
                    ot = osbpool.tile([P, S], F32, tag="osb", name=f"ot_{sc}")
                    nc.vector.tensor_add(ot[:], ps[:], bias["bo"][:])
                    nc.sync.dma_start(out_d[sc * P : (sc + 1) * P, :], ot[:])

                # Flat (h, kc) stream, PV lagging scores/exp by one tile so the
                # in-order PE never waits on a just-issued exp. After the last
                # PV of a head, the accumulator is copied to SBUF immediately to
                # free its PSUM bank; the norm chain reads the copy. o_proj
                # chunk k (needs heads 2k,2k+1 only) is emitted two heads later.
                from collections import deque
                pend = deque()
                pso_cur = None

                def pop_pv():
                    ph, pkc, ppso, pet = pend.popleft()
                    emit_pv(ph, pkc, ppso, pet)
                    if pkc == NCHUNK - 1:
                        emit_norm(ph, ppso)
                        if ph % 2 == 1 and ph >= 3:
                            emit_oproj((ph - 3) // 2)

                for h in range(H):
                    pso_cur = po.tile([P, S], F32, tag="po", name=f"pso_{h}")
                    for kc in range(NCHUNK):
                        et = emit_scores_exp(h, kc)
                        if len(pend) >= 4:
                            pop_pv()
                        pend.append((h, kc, pso_cur, et))
                while len(pend) > 1:
                    pop_pv()
                # last PV of head 15: slot o_proj(6) in front of the norm chain
                # so the PE stays busy while recip/bcast run on DVE/Pool.
                ph, pkc, ppso, pet = pend.popleft()
                emit_pv(ph, pkc, ppso, pet)
                emit_oproj(NCHUNK - 2)
                emit_norm(ph, ppso)
                emit_oproj(NCHUNK - 1)

    nc.compile()
    return nc


def _host_masks(prefix_b: int):
    """Combined multiplicative 0/1 mask, u8, applied to exp output.

    For scores-T tile kc (cols q in [kc*128, 1024)): element (i, q) keeps
    exp iff allowed(q, k=kc*128+i) = (q < prefix) or (k >= q).
    Segment kc occupies msk[:, off_kc : off_kc + (1024 - kc*128)].
    """
    i = np.arange(P)[:, None]
    segs = []
    for kc in range(NCHUNK):
        q = np.arange(kc * P, S)[None, :]
        k = kc * P + i
        allowed = (q < prefix_b) | (k >= q)
        segs.append(allowed.astype(np.uint8))
    return np.concatenate(segs, axis=1)


def kernel(x, prefix, Wq, bq, Wk, bk, Wv, bv, Wo, bo, _trace=False):
    x = np.asarray(x, dtype=np.float32)
    prefix = np.asarray(prefix)
    Wq, Wk, Wv, Wo = (np.ascontiguousarray(np.asarray(w, np.float32)) for w in (Wq, Wk, Wv, Wo))
    bv, bo = (
        np.broadcast_to(np.asarray(v, np.float32).reshape(1, D), (P, D)).copy()
        for v in (bv, bo)
    )
    bqk = np.stack(
        [np.asarray(bq, np.float32).reshape(NCHUNK, P), np.asarray(bk, np.float32).reshape(NCHUNK, P)], axis=0
    ).reshape(2 * NCHUNK, P).T.copy()  # [128, 16]: cols 0-7 = bq chunks, 8-15 = bk

    ones2d = np.ones((P, P), dtype=np.float32)
    if "nc" not in _CACHED:
        _CACHED["nc"] = build_nc()
    nc = _CACHED["nc"]

    in_maps = []
    for b in range(B):
        mask8 = _host_masks(int(prefix[b]))
        in_maps.append(
            {
                "xt": np.ascontiguousarray(x[b].T),
                "wq": Wq, "wk": Wk, "wv": Wv, "wo": Wo,
                "bqk": bqk, "bv": bv, "bo": bo, "ones2d": ones2d,
                "mask8": mask8,
            }
        )

    res = run_bass_kernel_spmd(nc, in_maps, core_ids=list(range(NCORES)), trace=_trace)
    out = np.stack([res.results[b]["out"] for b in range(B)], axis=0)
    if _trace:
        return out, res
    return out



# revision 32
# speedup vs baseline: 1.1342x; 1.1342x over previous
"""Trainium2 Bass kernel for nn_MultiHeadAttention_32031866093611.

Sharding: pure data parallel — batch b -> NeuronCore b (B == n_cores == 8).
Weights replicated. No collectives.

Per-core program (batch b, S=1024, D=1024, H=16, DK=64):

  qT[c]   = (Wq[:, c*128:+128]).T @ xT + bq       -> [128 d', 1024 s]  f32r
  kT[c]   = same with Wk                          -> [128 d', 1024 s]  f32r
  v[sc]   = (xT[:, sc*128:+128]).T @ Wv           -> [128 s, 16, 64+1] bf16
  per head h (c=h//2, r=h%2*64):
    sT[kc] = kT[c][r:r+64, kc*128:+128].T @ qT[c][r:r+64, :]  # [128 k, 1024 q]
    eT[kc] = exp(sT[kc]) -> bf16; 0/1 bf16 mask mult (DVE/Pool split)
    outT  += v[kc][:, h, :].T @ eT[kc]       # [65, 1024]: row 64 = denom
    copy outT -> sbuf bf16 (frees PSUM fast), recip+bcast+norm on the copy
  out[sc] = (attnT[.][:, sc*128:+128]).T @ Wo + bo  -> [128 s, 1024 d] -> DRAM

Engine balance: exp on ACT (~8.3us/head); DVE gets bf16 2x fast-mode ops
(masks on narrow segments, norm muls, recip, pso copy); Pool/GpSimd takes
the wide mask segments, partition_broadcast, and all bias adds.
"""

import numpy as np
import ml_dtypes

import concourse.bass as bass
import concourse.mybir as mybir
import concourse.tile as tile
from concourse import bacc
from concourse.bass_utils import run_bass_kernel_spmd

B, S, D, H = 8, 1024, 1024, 16
DK = D // H  # 64
P = 128
NCHUNK = S // P  # 8
NCORES = 8
F32R = mybir.dt.float32r
F32 = mybir.dt.float32
BF16 = mybir.dt.bfloat16
EXP = mybir.ActivationFunctionType.Exp
HALF = 512  # fp32 moving-operand max
# compact mask layout: cols [0,1024) = 8 per-kc diagonal blocks [P,128];
# cols [1024, 1920) = the shared below-diagonal column mask for q in [128,1024)
MSK_W = 1920
# all masks on DVE (bf16 2x mode keeps them ~0.1-0.5us each): Pool ops are
# ~2x slower and their queueing latency ends up blocking the first PV of
# each head. Pool keeps partition_broadcast + bias adds only.
POOL_KC = ()
KC_SEQ = list(range(NCHUNK))

_CACHED = {}

NWARM = 36


def build_nc(repeats=1):
    nc = bacc.Bacc("TRN2", target_bir_lowering=False, debug=False, num_devices=NCORES)

    xt_d = nc.dram_tensor("xt", [D, S], F32R, kind="ExternalInput").ap()
    wq_d = nc.dram_tensor("wq", [D, D], F32R, kind="ExternalInput").ap()
    wk_d = nc.dram_tensor("wk", [D, D], F32R, kind="ExternalInput").ap()
    wv_d = nc.dram_tensor("wv", [D, D], F32R, kind="ExternalInput").ap()
    wo_d = nc.dram_tensor("wo", [D, D], BF16, kind="ExternalInput").ap()
    bqk_d = nc.dram_tensor("bqk", [P, 2 * NCHUNK], F32, kind="ExternalInput").ap()
    ones_d = nc.dram_tensor("ones2d", [P, P], F32R, kind="ExternalInput").ap()
    bv_d = nc.dram_tensor("bv", [P, D], BF16, kind="ExternalInput").ap()
    bo_d = nc.dram_tensor("bo", [P, D], BF16, kind="ExternalInput").ap()
    msk_d = nc.dram_tensor("mask16", [P, MSK_W], BF16, kind="ExternalInput").ap()
    out_d = nc.dram_tensor("out", [S, D], BF16, kind="ExternalOutput").ap()

    with tile.TileContext(nc) as tc, nc.allow_low_precision(reason="bf16 attn"):
        with (
            tc.tile_pool(name="w", bufs=8) as wpool,
            tc.tile_pool(name="big", bufs=2) as bigpool,
            tc.tile_pool(name="qk", bufs=8) as qkpool,
            tc.tile_pool(name="v", bufs=8) as vpool,
            tc.tile_pool(name="cst", bufs=1) as cstpool,
            tc.tile_pool(name="exp", bufs=6) as exppool,
            tc.tile_pool(name="cp", bufs=2) as cppool,
            tc.tile_pool(name="rcp", bufs=1) as rcppool,
            tc.tile_pool(name="rbc", bufs=1) as rbcpool,
            tc.tile_pool(name="osb", bufs=1) as osbpool,
            tc.tile_pool(name="pp", bufs=2, space="PSUM") as pp,
            tc.tile_pool(name="po", bufs=2, space="PSUM") as po,
        ):
            for _rep in range(repeats):
                # ---- x chunks + Wq strips interleaved (fast PE start) ----
                # warm-up operand built by memset (no DMA dependency): PE can
                # start spinning its clock within ~0.3us of kernel start
                junk = cstpool.tile([P, 2 * P], BF16, tag="junk")
                nc.vector.memset(junk[:], 1.0)
                bqk = cstpool.tile([P, 2 * NCHUNK], F32, tag="bqk")
                ones2d = cstpool.tile([P, P], F32R, tag="ones2d")
                xtq = [
                    bigpool.tile([P, 4, S], F32R, tag="big", name=f"xtq_{g}")
                    for g in range(2)
                ]

                # Batched DMAs: each transfer moves a [512 dram rows, 512 col]
                # block into a [128, 4, 512] tile view (~1.46us, one 625ns DGE
                # pass) instead of 4 strip DMAs. Order follows consumption:
                # x+Wq-half0 first, then x-half1, then Wk-half0.
                def wdma(nm, w_dram, hf, dtype=F32R, gs=(0, 1)):
                    """Wcols [hf*512,(hf+1)*512) as 2 tiles [128, 4dc, 512]."""
                    ts = []
                    sl = slice(hf * HALF, (hf + 1) * HALF)
                    for g in gs:
                        t = wpool.tile([P, 4, HALF], dtype, tag="w", name=f"{nm}{hf}_{g}")
                        nc.sync.dma_start(
                            t[:],
                            w_dram[g * 4 * P : (g + 1) * 4 * P, sl].rearrange(
                                "(c p) j -> p c j", p=P
                            ),
                        )
                        ts.append(t)
                    return ts

                def xdma(g, jsl):
                    nc.sync.dma_start(
                        xtq[g][:, :, jsl],
                        xt_d[g * 4 * P : (g + 1) * 4 * P, jsl].rearrange(
                            "(c p) j -> p c j", p=P
                        ),
                    )

                # front of the DMA FIFO ordered by consumption: SP issues one
                # DMA per 650ns and DGE is serial, so nothing may queue ahead
                # of the data the first matmuls need. bqk (tiny, needed at the
                # first bias add ~17us) slots in after the first j0 data.
                xdma(0, slice(0, HALF))
                qh0 = []
                qh0 += wdma("wq", wq_d, 0, gs=(0,))
                xdma(0, slice(HALF, S))
                xdma(1, slice(0, HALF))
                qh0 += wdma("wq", wq_d, 0, gs=(1,))
                nc.sync.dma_start(bqk[:], bqk_d[:])
                xdma(1, slice(HALF, S))
                kh0 = wdma("wk", wk_d, 0)
                xt = [xtq[dc // 4][:, dc % 4, :] for dc in range(NCHUNK)]

                def wstrip(ts, dc):
                    return ts[dc // 4][:, dc % 4, :]

                # PE warm-up: throwaway matmuls on the memset tile keep the
                # HAM clock-gate busy while x/Wq stream in.
                wps = pp.tile([P, S], F32, tag="pp", name="warmup_ps")
                for wi in range(NWARM):
                    nc.tensor.matmul(
                        wps[:, 0 : 2 * P], junk[:, 0:P], junk[:], start=True, stop=True
                    )
                # masks/bv are needed only from the attention window on; their
                # DMAs are issued inside qk_proj so x/W strips go first
                msk = cstpool.tile([P, MSK_W], BF16, tag="msk")
                bias = {}
                # bv (v-proj) and bo (o_proj) lifetimes don't overlap: share slot
                bias["bv"] = cstpool.tile([P, D], BF16, tag="bvbo", name="bv_bc")

                # ---- helper: dense [d', s] projection (qT / kT) ----
                def proj_half(whalf_tiles, chalf, bcol0, out_tag, trickle=False):
                    """qT/kT chunks chalf*4 .. chalf*4+3 from one W column half.

                    trickle=True (first pair only): dc outermost so each
                    arriving W strip is consumed by both chunks at once while
                    the DMA stream is still priming. Everywhere else c stays
                    outermost so the bias-add of chunk c0 overlaps c1's
                    matmuls instead of stalling the PSUM rotation.
                    """
                    outs = []
                    if trickle:
                        # all 4 chunks at once (2 pp + 2 po accumulators):
                        # every arriving x/W batch unlocks 3.4us of matmuls,
                        # matching the ~2.9us/batch DMA rate from t~7us on
                        cs4 = [chalf * 4 + i for i in range(4)]
                        pss = {}
                        for i, c in enumerate(cs4):
                            pool = pp if i < 2 else po
                            pss[c] = pool.tile(
                                [P, S], F32, tag=("pp" if i < 2 else "po"),
                                name=f"ps_{out_tag}_{c}",
                            )
                        for dcs in (range(0, 4), range(4, 8)):
                            for j in range(2):
                                sl = slice(j * HALF, (j + 1) * HALF)
                                for dc in dcs:
                                    for c in cs4:
                                        lc = (c % 4) * P
                                        nc.tensor.matmul(
                                            pss[c][:, sl],
                                            wstrip(whalf_tiles, dc)[:, lc : lc + P],
                                            xt[dc][:, sl],
                                            start=(dc == 0),
                                            stop=(dc == NCHUNK - 1),
                                        )
                        for c in cs4:
                            o = qkpool.tile(
                                [P, S], F32R, tag=out_tag, name=f"{out_tag}_{c}"
                            )
                            nc.vector.tensor_add(
                                o[:],
                                pss[c][:],
                                bqk[:, bcol0 + c : bcol0 + c + 1].to_broadcast((P, S)),
                            )
                            outs.append(o)
                        return outs
                    for cp in range(2):
                        cs = (chalf * 4 + 2 * cp, chalf * 4 + 2 * cp + 1)
                        pss = {
                            c: pp.tile([P, S], F32, tag="pp", name=f"ps_{out_tag}_{c}")
                            for c in cs
                        }
                        for j in range(2):
                            sl = slice(j * HALF, (j + 1) * HALF)
                            for c in cs:
                                for dc in range(NCHUNK):
                                    lc = (c % 4) * P
                                    nc.tensor.matmul(
                                        pss[c][:, sl],
                                        wstrip(whalf_tiles, dc)[:, lc : lc + P],
                                        xt[dc][:, sl],
                                        start=(dc == 0),
                                        stop=(dc == NCHUNK - 1),
                                    )
                        for c in cs:
                            o = qkpool.tile(
                                [P, S], F32R, tag=out_tag, name=f"{out_tag}_{c}"
                            )
                            nc.vector.tensor_add(
                                o[:],
                                pss[c][:],
                                bqk[:, bcol0 + c : bcol0 + c + 1].to_broadcast((P, S)),
                            )
                            outs.append(o)
                    return outs

                with nc.named_scope("qk_proj"):
                    qh1 = wdma("wq", wq_d, 1)
                    qT = proj_half(qh0, 0, 0, "qT", trickle=True)
                    kh1 = wdma("wk", wk_d, 1)
                    kT = proj_half(kh0, 0, NCHUNK, "kT")
                    nc.sync.dma_start(msk[:], msk_d[:])
                    nc.sync.dma_start(bias["bv"][:], bv_d[:])
                    qT += proj_half(qh1, 1, 0, "qT")
                    kT += proj_half(kh1, 1, NCHUNK, "kT")

                # ---- v projection: [s, 16, 65] bf16 with ones column ----
                with nc.named_scope("v_proj"):
                    vh = [wdma("wv", wv_d, 0), wdma("wv", wv_d, 1)]
                    nc.sync.dma_start(ones2d[:], ones_d[:])
                    vtiles = []
                    for sc in range(NCHUNK):
                        # last chunks from po: their bias-adds must not block
                        # the first head's scores in the pp rotation
                        vpool_ps = po if sc >= NCHUNK - 2 else pp
                        ps = vpool_ps.tile([P, S], F32, tag=vpool_ps is po and "po" or "pp")
                        for j in range(2):
                            sl = slice(j * HALF, (j + 1) * HALF)
                            for dc in range(NCHUNK):
                                nc.tensor.matmul(
                                    ps[:, sl],
                                    xt[dc][:, sc * P : (sc + 1) * P],
                                    wstrip(vh[j], dc),
                                    start=(dc == 0),
                                    stop=(dc == NCHUNK - 1),
                                )
                        vt = vpool.tile([P, H, DK + 1], BF16, tag="v")
                        nc.vector.tensor_add(
                            vt[:, :, 0:DK],
                            ps[:].rearrange("p (h d) -> p h d", h=H),
                            bias["bv"][:].rearrange("p (h d) -> p h d", h=H),
                        )
                        nc.vector.tensor_copy(
                            vt[:, :, DK : DK + 1], ones2d[:, 0:1].to_broadcast((P, H, 1))
                        )
                        vtiles.append(vt)

                # ---- attention heads ----
                bias["bo"] = cstpool.tile([P, D], BF16, tag="bvbo", name="bo_bc")
                nc.sync.dma_start(bias["bo"][:], bo_d[:])
                attn = [None, None]

                # Wo strips (bf16) prefetched before the head loop.
                oh = [wdma("wo", wo_d, 0, dtype=BF16), wdma("wo", wo_d, 1, dtype=BF16)]

                def emit_scores_exp(h, kc):
                    """scores on PE, exp->bf16 on ACT, 0/1 mask mult DVE/Pool."""
                    c, r = h // 2, (h % 2) * DK
                    pss = pp.tile([P, S], F32, tag="pp", name=f"pss_{h}_{kc}")
                    lhs = kT[c][r : r + DK, kc * P : (kc + 1) * P]
                    for j in range(2):
                        sl = slice(j * HALF, (j + 1) * HALF)
                        nc.tensor.matmul(
                            pss[:, sl],
                            lhs,
                            qT[c][r : r + DK, sl],
                            start=True,
                            stop=True,
                        )
                    et = exppool.tile([P, S], BF16, tag="exp", name=f"et_{h}_{kc}")
                    nc.scalar.activation(et[:], pss[:], EXP)
                    # 0/1 mask: per-kc diag block + shared below-diag col mask
                    nc.vector.tensor_mul(
                        et[:, kc * P : (kc + 1) * P],
                        et[:, kc * P : (kc + 1) * P],
                        msk[:, kc * P : (kc + 1) * P],
                    )
                    if kc < NCHUNK - 1:
                        lo = (kc + 1) * P
                        nc.vector.tensor_mul(
                            et[:, lo:S], et[:, lo:S], msk[:, S - P + lo : S - P + S]
                        )
                    return et

                def emit_pv(h, kc, pos, pso, et):
                    for j in range(2):
                        sl = slice(j * HALF, (j + 1) * HALF)
                        nc.tensor.matmul(
                            pso[0 : DK + 1, sl],
                            vtiles[kc][:, h, :],
                            et[:, sl],
                            start=(pos == 0),
                            stop=(pos == NCHUNK - 1),
                        )

                def emit_norm(h, pso, last=False):
                    # copy accumulator to sbuf right away: frees the PSUM bank
                    # ~5us earlier than letting the norm chain read PSUM. For
                    # the last head freeing doesn't matter: skip the copy and
                    # shave its latency off the tail's critical path.
                    if last:
                        cpy = pso
                    else:
                        cpy = cppool.tile([DK + 1, S], BF16, tag="cp", name=f"cp_{h}")
                        nc.vector.tensor_copy(cpy[:], pso[0 : DK + 1, :])
                    rcp = rcppool.tile([1, S], BF16, tag="rcp", name=f"rcp_{h}")
                    nc.vector.reciprocal(rcp[:], cpy[DK : DK + 1, :])
                    rbc = rbcpool.tile([DK, S], BF16, tag="rbc", name=f"rbc_{h}")
                    nc.gpsimd.partition_broadcast(rbc[:], rcp[:])
                    # attn[g][e*64+d, cc, h*64+u] = O_h[u*16 + 2*(4g+cc) + e, d]/denom
                    src = cpy[0:DK, :].rearrange("d (u j) -> d j u", j=16)
                    rbs = rbc[:].rearrange("d (u j) -> d j u", j=16)
                    for g in range(2):
                        if attn[g] is None:
                            attn[g] = bigpool.tile(
                                [P, 4, S], BF16, tag="big", name=f"attnq_{g}"
                            )
                        for e in range(2):
                            jsl = slice(8 * g + e, 8 * (g + 1), 2)
                            nc.vector.tensor_mul(
                                attn[g][e * DK : (e + 1) * DK, :, h * DK : (h + 1) * DK],
                                src[:, jsl, :],
                                rbs[:, jsl, :],
                            )

                def emit_oproj(sc, bias_dve=True, split=False):
                    ps = po.tile([P, S], F32, tag="po", name=f"psf_{sc}")
                    ot = osbpool.tile([P, S], BF16, tag="osb", name=f"ot_{sc}")
                    eng = nc.vector
                    for j in range(2):
                        sl = slice(j * HALF, (j + 1) * HALF)
                        for cc in range(NCHUNK):
                            nc.tensor.matmul(
                                ps[:, sl],
                                attn[cc // 4][:, cc % 4, sc * P : (sc + 1) * P],
                                wstrip(oh[j], cc),
                                start=(cc == 0),
                                stop=(cc == NCHUNK - 1),
                            )
                        if split:
                            # tail: bias+DMA each half while the other half's
                            # matmuls still run on PE
                            eng.tensor_add(ot[:, sl], ps[:, sl], bias["bo"][:, sl])
                            nc.sync.dma_start(out_d[sc * P : (sc + 1) * P, sl], ot[:, sl])
                    if not split:
                        eng.tensor_add(ot[:], ps[:], bias["bo"][:])
                        nc.sync.dma_start(out_d[sc * P : (sc + 1) * P, :], ot[:])

                # Flat (h, kc) stream, PV lagging scores/exp so the in-order PE
                # never waits on a just-issued exp. o_proj chunk k (needs heads
                # 2k,2k+1 only) is emitted two heads later.
                from collections import deque
                pend = deque()
                pso_cur = None
                LAG = 5

                def pop_pv():
                    ph, pkc, pos, ppso, pet = pend.popleft()
                    emit_pv(ph, pkc, pos, ppso, pet)
                    if pos == NCHUNK - 1:
                        # o_proj first: its deps (norms of heads 2sc,2sc+1)
                        # are 2 heads old, so PE can start immediately instead
                        # of queueing behind this head's fresh norm muls
                        if ph % 2 == 1 and ph >= 3:
                            sc = (ph - 3) // 2
                            emit_oproj(sc, bias_dve=(sc % 2 == 0))
                        emit_norm(ph, ppso)

                for h in range(H):
                    pso_cur = po.tile([P, S], F32, tag="po", name=f"pso_{h}")
                    for pos, kc in enumerate(KC_SEQ):
                        et = emit_scores_exp(h, kc)
                        if len(pend) >= LAG:
                            pop_pv()
                        pend.append((h, kc, pos, pso_cur, et))
                while len(pend) > 1:
                    pop_pv()
                # last PV of head 15: slot o_proj(6) in front of the norm chain
                # so the PE stays busy while recip/bcast run on DVE/Pool; junk
                # matmuls after it keep the P-state hot through the norm chain.
                ph, pkc, pos, ppso, pet = pend.popleft()
                emit_pv(ph, pkc, pos, ppso, pet)
                emit_oproj(NCHUNK - 2, bias_dve=True)
                wps2 = pp.tile([P, S], F32, tag="pp", name="tail_junk_ps")
                for wi in range(10):
                    nc.tensor.matmul(
                        wps2[:, 0 : 2 * P], junk[:, 0:P], junk[:], start=True, stop=True
                    )
                emit_norm(ph, ppso, last=True)
                emit_oproj(NCHUNK - 1, bias_dve=True, split=True)

    nc.compile()
    return nc


def _host_masks(prefix_b: int):
    """Compact multiplicative 0/1 mask, bf16, applied to exp output.

    Cols [kc*128,(kc+1)*128): diag block kc: allowed(q=kc*128+j, k=kc*128+i)
    = (q < prefix) | (i >= j).  Cols [1024, 1920): shared column mask for
    q in [128, 1024): allowed = (q < prefix), broadcast over partitions
    (below the diagonal every k in the block is < q).
    """
    i = np.arange(P)[:, None]
    segs = []
    for kc in range(NCHUNK):
        q = np.arange(kc * P, (kc + 1) * P)[None, :]
        allowed = (q < prefix_b) | (i >= q - kc * P)
        segs.append(allowed.astype(ml_dtypes.bfloat16))
    q = np.arange(P, S)[None, :]
    colmask = np.broadcast_to(q < prefix_b, (P, S - P)).astype(ml_dtypes.bfloat16)
    segs.append(colmask)
    return np.concatenate(segs, axis=1)


def kernel(x, prefix, Wq, bq, Wk, bk, Wv, bv, Wo, bo, _trace=False):
    x = np.asarray(x, dtype=np.float32)
    prefix = np.asarray(prefix)
    Wq, Wk, Wv = (np.ascontiguousarray(np.asarray(w, np.float32)) for w in (Wq, Wk, Wv))
    Wo16 = np.ascontiguousarray(np.asarray(Wo, np.float32).astype(ml_dtypes.bfloat16))
    bv, bo = (
        np.broadcast_to(
            np.asarray(v, np.float32).astype(ml_dtypes.bfloat16).reshape(1, D), (P, D)
        ).copy()
        for v in (bv, bo)
    )
    bqk = np.stack(
        [np.asarray(bq, np.float32).reshape(NCHUNK, P), np.asarray(bk, np.float32).reshape(NCHUNK, P)], axis=0
    ).reshape(2 * NCHUNK, P).T.copy()  # [128, 16]: cols 0-7 = bq chunks, 8-15 = bk
    ones2d = np.ones((P, P), dtype=np.float32)
    if "nc" not in _CACHED:
        _CACHED["nc"] = build_nc()
    nc = _CACHED["nc"]

    in_maps = []
    for b in range(B):
        mask16 = _host_masks(int(prefix[b]))
        in_maps.append(
            {
                "xt": np.ascontiguousarray(x[b].T),
                "wq": Wq, "wk": Wk, "wv": Wv, "wo": Wo16,
                "bqk": bqk, "bv": bv, "bo": bo, "ones2d": ones2d,
                "mask16": mask16,
            }
        )

    res = run_bass_kernel_spmd(nc, in_maps, core_ids=list(range(NCORES)), trace=_trace)
    out = np.stack(
        [np.asarray(res.results[b]["out"]).astype(np.float32) for b in range(B)], axis=0
    )
    if _trace:
        return out, res
    return out


# revision 37
# speedup vs baseline: 1.1488x; 1.0129x over previous
"""Trainium2 Bass kernel for nn_MultiHeadAttention_32031866093611.

Sharding: pure data parallel — batch b -> NeuronCore b (B == n_cores == 8).
Weights replicated. No collectives.

Per-core program (batch b, S=1024, D=1024, H=16, DK=64):

  qT[c]   = (Wq[:, c*128:+128]).T @ xT + bq       -> [128 d', 1024 s]  f32r
  kT[c]   = same with Wk                          -> [128 d', 1024 s]  f32r
  v[sc]   = (xT[:, sc*128:+128]).T @ Wv           -> [128 s, 16, 64+1] bf16
  per head h (c=h//2, r=h%2*64):
    sT[kc] = kT[c][r:r+64, kc*128:+128].T @ qT[c][r:r+64, :]  # [128 k, 1024 q]
    eT[kc] = exp(sT[kc]) -> bf16; 0/1 bf16 mask mult (DVE/Pool split)
    outT  += v[kc][:, h, :].T @ eT[kc]       # [65, 1024]: row 64 = denom
    copy outT -> sbuf bf16 (frees PSUM fast), recip+bcast+norm on the copy
  out[sc] = (attnT[.][:, sc*128:+128]).T @ Wo + bo  -> [128 s, 1024 d] -> DRAM

Engine balance: exp on ACT (~8.3us/head); DVE gets bf16 2x fast-mode ops
(masks on narrow segments, norm muls, recip, pso copy); Pool/GpSimd takes
the wide mask segments, partition_broadcast, and all bias adds.
"""

import numpy as np
import ml_dtypes

import concourse.bass as bass
import concourse.mybir as mybir
import concourse.tile as tile
from concourse import bacc
from concourse.bass_utils import run_bass_kernel_spmd

B, S, D, H = 8, 1024, 1024, 16
DK = D // H  # 64
P = 128
NCHUNK = S // P  # 8
NCORES = 8
F32R = mybir.dt.float32r
F32 = mybir.dt.float32
BF16 = mybir.dt.bfloat16
EXP = mybir.ActivationFunctionType.Exp
HALF = 512  # fp32 moving-operand max
# compact mask layout: cols [0,1024) = 8 per-kc diagonal blocks [P,128];
# cols [1024, 1920) = the shared below-diagonal column mask for q in [128,1024)
MSK_W = 1920
# all masks on DVE (bf16 2x mode keeps them ~0.1-0.5us each): Pool ops are
# ~2x slower and their queueing latency ends up blocking the first PV of
# each head. Pool keeps partition_broadcast + bias adds only.
POOL_KC = ()
KC_SEQ = list(range(NCHUNK))

_CACHED = {}

NWARM = 36


def build_nc(repeats=1):
    nc = bacc.Bacc("TRN2", target_bir_lowering=False, debug=False, num_devices=NCORES)

    xt_d = nc.dram_tensor("xt", [D, S], F32R, kind="ExternalInput").ap()
    wq_d = nc.dram_tensor("wq", [D, D], F32R, kind="ExternalInput").ap()
    wk_d = nc.dram_tensor("wk", [D, D], F32R, kind="ExternalInput").ap()
    wv_d = nc.dram_tensor("wv", [D, D], F32R, kind="ExternalInput").ap()
    wo_d = nc.dram_tensor("wo", [D, D], BF16, kind="ExternalInput").ap()
    bqk_d = nc.dram_tensor("bqk", [P, 2 * NCHUNK], F32, kind="ExternalInput").ap()
    ones_d = nc.dram_tensor("ones2d", [P, P], F32R, kind="ExternalInput").ap()
    bv_d = nc.dram_tensor("bv", [P, D], BF16, kind="ExternalInput").ap()
    bo_d = nc.dram_tensor("bo", [P, D], BF16, kind="ExternalInput").ap()
    msk_d = nc.dram_tensor("mask16", [P, MSK_W], BF16, kind="ExternalInput").ap()
    out_d = nc.dram_tensor("out", [S, D], BF16, kind="ExternalOutput").ap()

    with tile.TileContext(nc) as tc, nc.allow_low_precision(reason="bf16 attn"):
        with (
            tc.tile_pool(name="w", bufs=8) as wpool,
            tc.tile_pool(name="big", bufs=2) as bigpool,
            tc.tile_pool(name="qk", bufs=8) as qkpool,
            tc.tile_pool(name="v", bufs=8) as vpool,
            tc.tile_pool(name="cst", bufs=1) as cstpool,
            tc.tile_pool(name="exp", bufs=6) as exppool,
            tc.tile_pool(name="cp", bufs=2) as cppool,
            tc.tile_pool(name="rcp", bufs=1) as rcppool,
            tc.tile_pool(name="rbc", bufs=1) as rbcpool,
            tc.tile_pool(name="osb", bufs=1) as osbpool,
            tc.tile_pool(name="pp", bufs=2, space="PSUM") as pp,
            tc.tile_pool(name="po", bufs=2, space="PSUM") as po,
        ):
            for _rep in range(repeats):
                # ---- x chunks + Wq strips interleaved (fast PE start) ----
                # warm-up operand built by memset (no DMA dependency): PE can
                # start spinning its clock within ~0.3us of kernel start
                junk = cstpool.tile([P, 2 * P], BF16, tag="junk")
                nc.vector.memset(junk[:], 1.0)
                bqk = cstpool.tile([P, 2 * NCHUNK], F32, tag="bqk")
                ones2d = cstpool.tile([P, P], F32R, tag="ones2d")
                xtq = [
                    bigpool.tile([P, 4, S], F32R, tag="big", name=f"xtq_{g}")
                    for g in range(2)
                ]

                # Batched DMAs: each transfer moves a [512 dram rows, 512 col]
                # block into a [128, 4, 512] tile view (~1.46us, one 625ns DGE
                # pass) instead of 4 strip DMAs. Order follows consumption:
                # x+Wq-half0 first, then x-half1, then Wk-half0.
                def wdma(nm, w_dram, hf, dtype=F32R, gs=(0, 1)):
                    """Wcols [hf*512,(hf+1)*512) as 2 tiles [128, 4dc, 512]."""
                    ts = []
                    sl = slice(hf * HALF, (hf + 1) * HALF)
                    for g in gs:
                        t = wpool.tile([P, 4, HALF], dtype, tag="w", name=f"{nm}{hf}_{g}")
                        nc.sync.dma_start(
                            t[:],
                            w_dram[g * 4 * P : (g + 1) * 4 * P, sl].rearrange(
                                "(c p) j -> p c j", p=P
                            ),
                        )
                        ts.append(t)
                    return ts

                def xdma(g, jsl):
                    nc.sync.dma_start(
                        xtq[g][:, :, jsl],
                        xt_d[g * 4 * P : (g + 1) * 4 * P, jsl].rearrange(
                            "(c p) j -> p c j", p=P
                        ),
                    )

                # front of the DMA FIFO ordered by consumption: SP issues one
                # DMA per 650ns and DGE is serial, so nothing may queue ahead
                # of the data the first matmuls need. bqk (tiny, needed at the
                # first bias add ~17us) slots in after the first j0 data.
                xdma(0, slice(0, HALF))
                qh0 = []
                qh0 += wdma("wq", wq_d, 0, gs=(0,))
                xdma(0, slice(HALF, S))
                xdma(1, slice(0, HALF))
                qh0 += wdma("wq", wq_d, 0, gs=(1,))
                nc.sync.dma_start(bqk[:], bqk_d[:])
                xdma(1, slice(HALF, S))
                kh0 = wdma("wk", wk_d, 0)
                xt = [xtq[dc // 4][:, dc % 4, :] for dc in range(NCHUNK)]

                def wstrip(ts, dc):
                    return ts[dc // 4][:, dc % 4, :]

                # PE warm-up: throwaway matmuls on the memset tile keep the
                # HAM clock-gate busy while x/Wq stream in.
                wps = pp.tile([P, S], F32, tag="pp", name="warmup_ps")
                for wi in range(NWARM):
                    nc.tensor.matmul(
                        wps[:, 0 : 2 * P], junk[:, 0:P], junk[:], start=True, stop=True
                    )
                # masks/bv are needed only from the attention window on; their
                # DMAs are issued inside qk_proj so x/W strips go first
                msk = cstpool.tile([P, MSK_W], BF16, tag="msk")
                bias = {}
                # bv (v-proj) and bo (o_proj) lifetimes don't overlap: share slot
                bias["bv"] = cstpool.tile([P, D], BF16, tag="bvbo", name="bv_bc")

                # ---- helper: dense [d', s] projection (qT / kT) ----
                def proj_half(whalf_tiles, chalf, bcol0, out_tag, trickle=False):
                    """qT/kT chunks chalf*4 .. chalf*4+3 from one W column half.

                    trickle=True (first pair only): dc outermost so each
                    arriving W strip is consumed by both chunks at once while
                    the DMA stream is still priming. Everywhere else c stays
                    outermost so the bias-add of chunk c0 overlaps c1's
                    matmuls instead of stalling the PSUM rotation.
                    """
                    outs = []
                    if trickle:
                        # all 4 chunks at once (2 pp + 2 po accumulators):
                        # every arriving x/W batch unlocks 3.4us of matmuls,
                        # matching the ~2.9us/batch DMA rate from t~7us on
                        cs4 = [chalf * 4 + i for i in range(4)]
                        pss = {}
                        for i, c in enumerate(cs4):
                            pool = pp if i < 2 else po
                            pss[c] = pool.tile(
                                [P, S], F32, tag=("pp" if i < 2 else "po"),
                                name=f"ps_{out_tag}_{c}",
                            )
                        for dcs in (range(0, 4), range(4, 8)):
                            for j in range(2):
                                sl = slice(j * HALF, (j + 1) * HALF)
                                for dc in dcs:
                                    for c in cs4:
                                        lc = (c % 4) * P
                                        nc.tensor.matmul(
                                            pss[c][:, sl],
                                            wstrip(whalf_tiles, dc)[:, lc : lc + P],
                                            xt[dc][:, sl],
                                            start=(dc == 0),
                                            stop=(dc == NCHUNK - 1),
                                        )
                        for c in cs4:
                            o = qkpool.tile(
                                [P, S], F32R, tag=out_tag, name=f"{out_tag}_{c}"
                            )
                            nc.vector.tensor_add(
                                o[:],
                                pss[c][:],
                                bqk[:, bcol0 + c : bcol0 + c + 1].to_broadcast((P, S)),
                            )
                            outs.append(o)
                        return outs
                    for cp in range(2):
                        cs = (chalf * 4 + 2 * cp, chalf * 4 + 2 * cp + 1)
                        pss = {
                            c: pp.tile([P, S], F32, tag="pp", name=f"ps_{out_tag}_{c}")
                            for c in cs
                        }
                        for j in range(2):
                            sl = slice(j * HALF, (j + 1) * HALF)
                            for c in cs:
                                for dc in range(NCHUNK):
                                    lc = (c % 4) * P
                                    nc.tensor.matmul(
                                        pss[c][:, sl],
                                        wstrip(whalf_tiles, dc)[:, lc : lc + P],
                                        xt[dc][:, sl],
                                        start=(dc == 0),
                                        stop=(dc == NCHUNK - 1),
                                    )
                        for c in cs:
                            o = qkpool.tile(
                                [P, S], F32R, tag=out_tag, name=f"{out_tag}_{c}"
                            )
                            nc.vector.tensor_add(
                                o[:],
                                pss[c][:],
                                bqk[:, bcol0 + c : bcol0 + c + 1].to_broadcast((P, S)),
                            )
                            outs.append(o)
                    return outs

                with nc.named_scope("qk_proj"):
                    qh1 = wdma("wq", wq_d, 1)
                    qT = proj_half(qh0, 0, 0, "qT", trickle=True)
                    kh1 = wdma("wk", wk_d, 1)
                    kT = proj_half(kh0, 0, NCHUNK, "kT")
                    nc.sync.dma_start(msk[:], msk_d[:])
                    nc.sync.dma_start(bias["bv"][:], bv_d[:])
                    qT += proj_half(qh1, 1, 0, "qT")
                    kT += proj_half(kh1, 1, NCHUNK, "kT")

                from collections import deque
                pend = deque()
                psos = {}
                LAG = 5

                def emit_scores_exp(h, kc):
                    """scores on PE, exp->bf16 on ACT, 0/1 mask mult on DVE."""
                    c, r = h // 2, (h % 2) * DK
                    pss = pp.tile([P, S], F32, tag="pp", name=f"pss_{h}_{kc}")
                    lhs = kT[c][r : r + DK, kc * P : (kc + 1) * P]
                    for j in range(2):
                        sl = slice(j * HALF, (j + 1) * HALF)
                        nc.tensor.matmul(
                            pss[:, sl],
                            lhs,
                            qT[c][r : r + DK, sl],
                            start=True,
                            stop=True,
                        )
                    et = exppool.tile([P, S], BF16, tag="exp", name=f"et_{h}_{kc}")
                    nc.scalar.activation(et[:], pss[:], EXP)
                    # 0/1 mask: per-kc diag block + shared below-diag col mask
                    nc.vector.tensor_mul(
                        et[:, kc * P : (kc + 1) * P],
                        et[:, kc * P : (kc + 1) * P],
                        msk[:, kc * P : (kc + 1) * P],
                    )
                    if kc < NCHUNK - 1:
                        lo = (kc + 1) * P
                        nc.vector.tensor_mul(
                            et[:, lo:S], et[:, lo:S], msk[:, S - P + lo : S - P + S]
                        )
                    return et

                # ---- v projection: [s, 16, 65] bf16 with ones column ----
                with nc.named_scope("v_proj"):
                    vh = [wdma("wv", wv_d, 0), wdma("wv", wv_d, 1)]
                    nc.sync.dma_start(ones2d[:], ones_d[:])
                    vtiles = []
                    for sc in range(NCHUNK):
                        # last chunks from po: their bias-adds must not block
                        # the first head's scores in the pp rotation
                        vpool_ps = po if sc >= NCHUNK - 2 else pp
                        ps = vpool_ps.tile([P, S], F32, tag=vpool_ps is po and "po" or "pp")
                        for j in range(2):
                            sl = slice(j * HALF, (j + 1) * HALF)
                            for dc in range(NCHUNK):
                                nc.tensor.matmul(
                                    ps[:, sl],
                                    xt[dc][:, sc * P : (sc + 1) * P],
                                    wstrip(vh[j], dc),
                                    start=(dc == 0),
                                    stop=(dc == NCHUNK - 1),
                                )
                        vt = vpool.tile([P, H, DK + 1], BF16, tag="v")
                        nc.vector.tensor_add(
                            vt[:, :, 0:DK],
                            ps[:].rearrange("p (h d) -> p h d", h=H),
                            bias["bv"][:].rearrange("p (h d) -> p h d", h=H),
                        )
                        nc.vector.tensor_copy(
                            vt[:, :, DK : DK + 1], ones2d[:, 0:1].to_broadcast((P, H, 1))
                        )
                        vtiles.append(vt)
                        if sc >= 2:
                            # warm the attention window: head-0 scores/exp
                            # interleave with v chunks (no v dependency)
                            et = emit_scores_exp(0, sc - 2)
                            pend.append((0, sc - 2, sc - 2, et))

                # ---- attention heads ----
                bias["bo"] = cstpool.tile([P, D], BF16, tag="bvbo", name="bo_bc")
                nc.sync.dma_start(bias["bo"][:], bo_d[:])
                attn = [None, None]

                # Wo strips (bf16) prefetched before the head loop.
                oh = [wdma("wo", wo_d, 0, dtype=BF16), wdma("wo", wo_d, 1, dtype=BF16)]

                def emit_pv(h, kc, pos, pso, et):
                    for j in range(2):
                        sl = slice(j * HALF, (j + 1) * HALF)
                        nc.tensor.matmul(
                            pso[0 : DK + 1, sl],
                            vtiles[kc][:, h, :],
                            et[:, sl],
                            start=(pos == 0),
                            stop=(pos == NCHUNK - 1),
                        )

                def emit_norm(h, pso, last=False):
                    # copy accumulator to sbuf right away: frees the PSUM bank
                    # ~5us earlier than letting the norm chain read PSUM. For
                    # the last head freeing doesn't matter: skip the copy and
                    # shave its latency off the tail's critical path.
                    if last:
                        cpy = pso
                    else:
                        cpy = cppool.tile([DK + 1, S], BF16, tag="cp", name=f"cp_{h}")
                        nc.vector.tensor_copy(cpy[:], pso[0 : DK + 1, :])
                    rcp = rcppool.tile([1, S], BF16, tag="rcp", name=f"rcp_{h}")
                    nc.vector.reciprocal(rcp[:], cpy[DK : DK + 1, :])
                    rbc = rbcpool.tile([DK, S], BF16, tag="rbc", name=f"rbc_{h}")
                    nc.gpsimd.partition_broadcast(rbc[:], rcp[:])
                    # attn[g][e*64+d, cc, h*64+u] = O_h[u*16 + 2*(4g+cc) + e, d]/denom
                    src = cpy[0:DK, :].rearrange("d (u j) -> d j u", j=16)
                    rbs = rbc[:].rearrange("d (u j) -> d j u", j=16)
                    for g in range(2):
                        if attn[g] is None:
                            attn[g] = bigpool.tile(
                                [P, 4, S], BF16, tag="big", name=f"attnq_{g}"
                            )
                        for e in range(2):
                            jsl = slice(8 * g + e, 8 * (g + 1), 2)
                            nc.vector.tensor_mul(
                                attn[g][e * DK : (e + 1) * DK, :, h * DK : (h + 1) * DK],
                                src[:, jsl, :],
                                rbs[:, jsl, :],
                            )

                def emit_oproj(sc, bias_dve=True, split=False):
                    ps = po.tile([P, S], F32, tag="po", name=f"psf_{sc}")
                    ot = osbpool.tile([P, S], BF16, tag="osb", name=f"ot_{sc}")
                    eng = nc.vector
                    for j in range(2):
                        sl = slice(j * HALF, (j + 1) * HALF)
                        for cc in range(NCHUNK):
                            nc.tensor.matmul(
                                ps[:, sl],
                                attn[cc // 4][:, cc % 4, sc * P : (sc + 1) * P],
                                wstrip(oh[j], cc),
                                start=(cc == 0),
                                stop=(cc == NCHUNK - 1),
                            )
                        if split:
                            # tail: bias+DMA each half while the other half's
                            # matmuls still run on PE
                            eng.tensor_add(ot[:, sl], ps[:, sl], bias["bo"][:, sl])
                            nc.sync.dma_start(out_d[sc * P : (sc + 1) * P, sl], ot[:, sl])
                    if not split:
                        eng.tensor_add(ot[:], ps[:], bias["bo"][:])
                        nc.sync.dma_start(out_d[sc * P : (sc + 1) * P, :], ot[:])

                # Flat (h, kc) stream, PV lagging scores/exp so the in-order PE
                # never waits on a just-issued exp. o_proj chunk k (needs heads
                # 2k,2k+1 only) is emitted two heads later.
                def pop_pv():
                    ph, pkc, pos, pet = pend.popleft()
                    if pos == 0:
                        psos[ph] = po.tile([P, S], F32, tag="po", name=f"pso_{ph}")
                    emit_pv(ph, pkc, pos, psos[ph], pet)
                    if pos == NCHUNK - 1:
                        # o_proj first: its deps (norms of heads 2sc,2sc+1)
                        # are 2 heads old, so PE can start immediately instead
                        # of queueing behind this head's fresh norm muls
                        if ph % 2 == 1 and ph >= 3:
                            sc = (ph - 3) // 2
                            emit_oproj(sc, bias_dve=(sc % 2 == 0))
                        emit_norm(ph, psos.pop(ph))

                for h in range(H):
                    for pos, kc in enumerate(KC_SEQ):
                        if h == 0 and pos < NCHUNK - 2:
                            continue  # emitted in the v_proj prologue
                        et = emit_scores_exp(h, kc)
                        if len(pend) >= LAG:
                            pop_pv()
                        pend.append((h, kc, pos, et))
                while len(pend) > 1:
                    pop_pv()
                # last PV of head 15: slot o_proj(6) in front of the norm chain
                # so the PE stays busy while recip/bcast run on DVE/Pool; junk
                # matmuls after it keep the P-state hot through the norm chain.
                ph, pkc, pos, pet = pend.popleft()
                emit_pv(ph, pkc, pos, psos[ph], pet)
                ppso = psos.pop(ph)
                emit_oproj(NCHUNK - 2, bias_dve=True)
                wps2 = pp.tile([P, S], F32, tag="pp", name="tail_junk_ps")
                for wi in range(10):
                    nc.tensor.matmul(
                        wps2[:, 0 : 2 * P], junk[:, 0:P], junk[:], start=True, stop=True
                    )
                emit_norm(ph, ppso, last=True)
                emit_oproj(NCHUNK - 1, bias_dve=True, split=True)

    nc.compile()
    return nc


def _host_masks(prefix_b: int):
    """Compact multiplicative 0/1 mask, bf16, applied to exp output.

    Cols [kc*128,(kc+1)*128): diag block kc: allowed(q=kc*128+j, k=kc*128+i)
    = (q < prefix) | (i >= j).  Cols [1024, 1920): shared column mask for
    q in [128, 1024): allowed = (q < prefix), broadcast over partitions
    (below the diagonal every k in the block is < q).
    """
    i = np.arange(P)[:, None]
    segs = []
    for kc in range(NCHUNK):
        q = np.arange(kc * P, (kc + 1) * P)[None, :]
        allowed = (q < prefix_b) | (i >= q - kc * P)
        segs.append(allowed.astype(ml_dtypes.bfloat16))
    q = np.arange(P, S)[None, :]
    colmask = np.broadcast_to(q < prefix_b, (P, S - P)).astype(ml_dtypes.bfloat16)
    segs.append(colmask)
    return np.concatenate(segs, axis=1)


def kernel(x, prefix, Wq, bq, Wk, bk, Wv, bv, Wo, bo, _trace=False):
    x = np.asarray(x, dtype=np.float32)
    prefix = np.asarray(prefix)
    Wq, Wk, Wv = (np.ascontiguousarray(np.asarray(w, np.float32)) for w in (Wq, Wk, Wv))
    Wo16 = np.ascontiguousarray(np.asarray(Wo, np.float32).astype(ml_dtypes.bfloat16))
    bv, bo = (
        np.broadcast_to(
            np.asarray(v, np.float32).astype(ml_dtypes.bfloat16).reshape(1, D), (P, D)
        ).copy()
        for v in (bv, bo)
    )
    bqk = np.stack(
        [np.asarray(bq, np.float32).reshape(NCHUNK, P), np.asarray(bk, np.float32).reshape(NCHUNK, P)], axis=0
    ).reshape(2 * NCHUNK, P).T.copy()  # [128, 16]: cols 0-7 = bq chunks, 8-15 = bk
    ones2d = np.ones((P, P), dtype=np.float32)
    if "nc" not in _CACHED:
        _CACHED["nc"] = build_nc()
    nc = _CACHED["nc"]

    in_maps = []
    for b in range(B):
        mask16 = _host_masks(int(prefix[b]))
        in_maps.append(
            {
                "xt": np.ascontiguousarray(x[b].T),
                "wq": Wq, "wk": Wk, "wv": Wv, "wo": Wo16,
                "bqk": bqk, "bv": bv, "bo": bo, "ones2d": ones2d,
                "mask16": mask16,
            }
        )

    res = run_bass_kernel_spmd(nc, in_maps, core_ids=list(range(NCORES)), trace=_trace)
    out = np.stack(
        [np.asarray(res.results[b]["out"]).astype(np.float32) for b in range(B)], axis=0
    )
    if _trace:
        return out, res
    return out
